# revision 17
# baseline (speedup 1.0000x reference)
"""Trainium2 Bass kernel for nn_DirectedGATLayer (GNN message passing).

Sharding: nodes are partitioned across 8 cores by receiver; each of the two
attention directions routes every edge to the core owning its receiver, so
segment softmax + scatter-add are core-local (no collectives).

Launch 1 (sharded): each core projects its own nodes: h@W, h@W_self and all
16 attention-logit dot products (via host-precomputed W@A columns) in one
matmul, emitting a bf16 node-table row [s_in_src(4)|s_out_src(4)|4x(32f,1.0)]
padded to 512 B, plus s_dst tables and h_self.

Host concatenates the table shards (replicated node table).

Launch 2: receivers are grouped into blocks (<=128 nodes); per block and
direction the core bulk-gathers sender rows with dma_gather (int16 indices,
4 x 25088-row windows of the table, negative indices = skipped padding),
gathers receiver s_dst rows, computes alpha = exp(leakyrelu(s_src + s_dst))
(softmax shift taken as 0 — exactly cancels in the normalization), scales
features by alpha, builds a one-hot edge->slot matrix with one is_equal
against an iota, and segment-sums via one matmul per 128-edge subtile into
PSUM [slot, 4*(32+1)] (numerator + denominator together).  The epilogue
divides by (denom+eps), adds the other direction, h_self and bias, applies
LayerNorm (sqrt via exp(0.5*ln) to stay in one ACT table), and
dma_scatter_adds rows into the zero-initialized output.
"""

import os
import sys
import numpy as np

for _p in ("/opt/trn_rl_repo", "/root/.axon_site/_ro/trn_rl_repo"):
    if os.path.isdir(_p) and _p not in sys.path:
        sys.path.insert(0, _p)

import jax  # noqa: E402

_CACHE_DIR = os.environ.get("KERNEL_JAX_CACHE", "/tmp/gat_jax_cache")
try:
    os.makedirs(_CACHE_DIR, exist_ok=True)
    jax.config.update("jax_compilation_cache_dir", _CACHE_DIR)
    jax.config.update("jax_persistent_cache_min_compile_time_secs", 0.0)
    jax.config.update("jax_persistent_cache_min_entry_size_bytes", 0)
except Exception:
    pass

import ml_dtypes  # noqa: E402
import concourse.bacc as bacc  # noqa: E402
import concourse.bass as bass  # noqa: E402
import concourse.mybir as mybir  # noqa: E402
from concourse.bass_utils import run_bass_kernel_spmd  # noqa: E402
from concourse.masks import make_identity  # noqa: E402
from concourse.tile import TileContext  # noqa: E402
from contextlib import ExitStack  # noqa: E402

BF16 = mybir.dt.bfloat16
F32 = mybir.dt.float32
I16 = mybir.dt.int16
NPBF16 = ml_dtypes.bfloat16

AF = mybir.ActivationFunctionType
ALU = mybir.AluOpType


class Cfg:
    def __init__(self, N=100000, E=1600000, IN_DIM=128, OUT_DIM=128, H=4,
                 n_cores=8, S=20, neg_slope=0.2, eps_sm=1e-8, eps_ln=1e-5):
        self.N, self.E, self.IN, self.OUT, self.H = N, E, IN_DIM, OUT_DIM, H
        self.HD = OUT_DIM // H
        self.NC = n_cores
        self.NEG = neg_slope
        self.EPS_SM = eps_sm
        self.EPS_LN = eps_ln
        assert N % n_cores == 0
        self.NPC = N // n_cores
        self.NPC_PAD = ((self.NPC + 127) // 128) * 128
        if self.NPC_PAD == self.NPC:
            self.NPC_PAD += 128
        self.N_TBL = n_cores * self.NPC_PAD
        self.TROW = 256                  # bf16 cols -> 512B rows
        self.FCOL = 2 * H                # feature cols start (8)
        self.DUMP = self.NPC_PAD - 1
        self.WROW = 25088                # int16 window (4*25088 >= N_TBL)
        self.NW = min(4, (self.N_TBL + self.WROW - 1) // self.WROW)
        assert self.N_TBL <= self.NW * self.WROW
        assert self.WROW + self.WROW // 4 < 32768
        self.SW = [S // self.NW] * self.NW
        for i in range(S % self.NW):
            self.SW[i] += 1
        self.S = S
        self.G = 2                       # blocks per batch
        self.G2 = 8                      # blocks per hself/out group
        self.WOFF = np.cumsum([0] + [w * self.G for w in self.SW]).tolist()
        self.WA_COLS = 2 * self.OUT + 4 * H
        self.FF_BATCH = 4                # first batches: no negative padding


DEFAULT_CFG = Cfg()

_STAGE = int(os.environ.get("P2STAGE", "99"))

_PROG_CACHE = {}


def _wrap16(idx):
    """[n] int -> [128, n//16] int16 wrapped + replicated across Q7 cores."""
    n = len(idx)
    assert n % 16 == 0
    w = np.empty((128, n // 16), np.int16)
    j = np.arange(n)
    v = idx.astype(np.int16)
    for cc in range(8):
        w[16 * cc + (j % 16), j // 16] = v
    return w


def _build_wa(cfg, W, W_self, a_in, a_out):
    H, HD = cfg.H, cfg.HD
    P = np.zeros((cfg.OUT, 4 * H), np.float32)
    for h in range(H):
        rows = slice(h * HD, (h + 1) * HD)
        P[rows, 0 * H + h] = a_in[h, :HD]
        P[rows, 1 * H + h] = a_in[h, HD:]
        P[rows, 2 * H + h] = a_out[h, :HD]
        P[rows, 3 * H + h] = a_out[h, HD:]
    WA = np.concatenate([W, W_self, W @ P], axis=1).astype(np.float32)
    return np.ascontiguousarray(WA.astype(NPBF16))


def _preprocess(cfg, edge_index):
    """Per-core streams for phase 2. Returns B and per-core dicts."""
    src, dst = np.asarray(edge_index[0]), np.asarray(edge_index[1])
    NW, G, S = cfg.NW, cfg.G, cfg.S
    percore = []
    for c in range(cfg.NC):
        lo, hi = c * cfg.NPC, (c + 1) * cfg.NPC
        dirs = []
        for snd, rcv in ((src, dst), (dst, src)):
            m = (rcv >= lo) & (rcv < hi)
            r = (rcv[m] - lo).astype(np.int64)
            s = snd[m].astype(np.int64)
            trow = (s // cfg.NPC) * cfg.NPC_PAD + (s % cfg.NPC)
            w = trow // cfg.WROW
            # per (receiver, window) degree
            degw = np.zeros((cfg.NPC, NW), np.int64)
            np.add.at(degw, (r, w), 1)
            dirs.append((r, trow, w, degw))
        percore.append(dirs)

    # greedy receiver blocking (shared across dirs)
    blocks_all = []
    B = 0
    for c in range(cfg.NC):
        caps = np.array(cfg.SW) * 128
        cur = np.zeros((2, NW), np.int64)
        n_cur = 0
        starts = [0]
        for r in range(cfg.NPC):
            need = [percore[c][d][3][r] for d in range(2)]
            over = any((cur[d] + need[d] > caps).any() for d in range(2))
            if n_cur == 128 or (over and n_cur > 0):
                starts.append(r)
                cur[:] = 0
                n_cur = 0
            cur[0] += need[0]
            cur[1] += need[1]
            n_cur += 1
        st = np.asarray(starts, np.int64)
        en = np.append(st[1:], cfg.NPC)
        blocks_all.append((st, en))
        B = max(B, len(st))
    B = ((B + cfg.G2 - 1) // cfg.G2) * cfg.G2

    streams = []
    for c in range(cfg.NC):
        st, en = blocks_all[c]
        Bc = len(st)
        blk_of_r = np.zeros(cfg.NPC, np.int64)
        for b in range(Bc):
            blk_of_r[st[b]:en[b]] = b
        slot_of_r = np.arange(cfg.NPC, dtype=np.int64) - st[blk_of_r]

        n_dir = B * S * 128
        midx = np.full(2 * n_dir, -1, np.int64)
        sidx = np.full(2 * n_dir, -1, np.int64)
        slots = np.full(2 * n_dir, 200.0, np.float32)
        for d in range(2):
            r, trow, w, _ = percore[c][d]
            b_e = blk_of_r[r]
            slot_e = slot_of_r[r]
            i_e = b_e // G
            blk_e = b_e % G
            # position of each edge within its (block, window) run
            key = b_e * NW + w
            order = np.argsort(key, kind="stable")
            ks = key[order]
            run_start = np.searchsorted(ks, np.arange(Bc * NW))
            k_in_run = np.empty(len(r), np.int64)
            k_in_run[order] = np.arange(len(r)) - run_start[ks]
            capw = np.array(cfg.SW) * 128
            assert (k_in_run < capw[w]).all(), "window run overflow"
            woff = np.asarray(cfg.WOFF)
            pos = (d * B * S + i_e * G * S) * 128 \
                + woff[w] * 128 + blk_e * capw[w] + k_in_run
            midx[pos] = trow - w * cfg.WROW
            sidx[pos] = r
            slots[pos] = slot_e
        # first FF_BATCH batches per dir: replace -1 padding with idx 0
        for d in range(2):
            a = d * n_dir
            z = slice(a, a + cfg.FF_BATCH * G * S * 128)
            mz = midx[z]
            mz[mz < 0] = 0
            midx[z] = mz
            sz = sidx[z]
            sz[sz < 0] = 0
            sidx[z] = sz

        pvec = np.arange(128, dtype=np.int64)
        starts_pad = np.full(B, cfg.NPC_PAD, np.int64)
        ends_pad = np.zeros(B, np.int64)
        starts_pad[:Bc] = st
        ends_pad[:Bc] = en
        rows = starts_pad[None, :] + pvec[:, None]
        valid = rows < ends_pad[None, :]
        hrows = np.where(valid, rows, cfg.DUMP)        # [128, B]
        hflat = hrows.T.reshape(-1)                    # position b*128+p
        streams.append(dict(
            midx=_wrap16(midx), sidx=_wrap16(sidx),
            slots=slots.astype(NPBF16).reshape(2 * B * S, 128).T.copy(),
            hoidx=_wrap16(hflat), ooidx=_wrap16(hflat)))
    return B, streams


# --------------------------------------------------------------------------
# phase 1
# --------------------------------------------------------------------------

def _build_phase1(cfg):
    H, HD, OUT = cfg.H, cfg.HD, cfg.OUT
    T = cfg.NPC_PAD // 128
    nc = bacc.Bacc("TRN2", target_bir_lowering=False, debug=False,
                   num_devices=cfg.NC)
    h_in = nc.dram_tensor("h", [cfg.NPC_PAD, cfg.IN], F32,
                          kind="ExternalInput")
    wa_in = nc.dram_tensor("wa", [cfg.IN, cfg.WA_COLS], BF16,
                           kind="ExternalInput")
    tbl_o = nc.dram_tensor("tbl", [cfg.NPC_PAD, cfg.TROW], BF16,
                           kind="ExternalOutput")
    sdi_o = nc.dram_tensor("sdst_in", [cfg.NPC_PAD, 128], BF16,
                           kind="ExternalOutput")
    sdo_o = nc.dram_tensor("sdst_out", [cfg.NPC_PAD, 128], BF16,
                           kind="ExternalOutput")
    hs_o = nc.dram_tensor("hself", [cfg.NPC_PAD, OUT], F32,
                          kind="ExternalOutput")

    with TileContext(nc) as tc, ExitStack() as ctx:
        cpool = ctx.enter_context(tc.tile_pool(name="const", bufs=1))
        wpool = ctx.enter_context(tc.tile_pool(name="work", bufs=3))
        ppool = ctx.enter_context(tc.tile_pool(name="psum", bufs=2,
                                               space="PSUM"))
        ident = cpool.tile([128, 128], BF16)
        make_identity(nc, ident[:])
        wa_sb = cpool.tile([cfg.IN, cfg.WA_COLS], BF16)
        nc.sync.dma_start(out=wa_sb[:], in_=wa_in.ap())

        for t in range(T):
            rows = slice(t * 128, (t + 1) * 128)
            hbf = wpool.tile([128, cfg.IN], BF16)
            nc.gpsimd.dma_start(out=hbf[:], in_=h_in.ap()[rows, :])  # cast
            psT = ppool.tile([128, 128], BF16, tag="psT")
            nc.tensor.transpose(out=psT[:], in_=hbf[:], identity=ident[:])
            hT = wpool.tile([128, 128], BF16)
            nc.scalar.activation(out=hT[:], in_=psT[:], func=AF.Copy)
            ps = ppool.tile([128, cfg.WA_COLS], F32, tag="psP")
            nc.tensor.matmul(out=ps[:], lhsT=hT[:], rhs=wa_sb[:],
                             start=True, stop=True)

            ttile = wpool.tile([128, cfg.TROW], BF16)
            nc.gpsimd.memset(ttile[:], 1.0)
            nc.vector.tensor_copy(out=ttile[:, 0:H],
                                  in_=ps[:, 2 * OUT:2 * OUT + H])
            nc.vector.tensor_copy(out=ttile[:, H:2 * H],
                                  in_=ps[:, 2 * OUT + 2 * H:2 * OUT + 3 * H])
            feat_dst = ttile[:, cfg.FCOL:cfg.FCOL + H * (HD + 1)]
            feat_dst = feat_dst.rearrange("p (h k) -> p h k", h=H)[:, :, 0:HD]
            feat_src = ps[:, 0:OUT].rearrange("p (h k) -> p h k", h=H)
            nc.vector.tensor_copy(out=feat_dst, in_=feat_src)

            sdi = wpool.tile([128, 128], BF16)
            nc.gpsimd.memset(sdi[:], 0.0)
            nc.vector.tensor_copy(out=sdi[:, 0:H],
                                  in_=ps[:, 2 * OUT + H:2 * OUT + 2 * H])
            sdo = wpool.tile([128, 128], BF16)
            nc.gpsimd.memset(sdo[:], 0.0)
            nc.vector.tensor_copy(out=sdo[:, 0:H],
                                  in_=ps[:, 2 * OUT + 3 * H:2 * OUT + 4 * H])
            hs = wpool.tile([128, OUT], F32)
            nc.scalar.activation(out=hs[:], in_=ps[:, OUT:2 * OUT],
                                 func=AF.Copy)

            nc.sync.dma_start(out=tbl_o.ap()[rows, :], in_=ttile[:])
            nc.sync.dma_start(out=sdi_o.ap()[rows, :], in_=sdi[:])
            nc.sync.dma_start(out=sdo_o.ap()[rows, :], in_=sdo[:])
            nc.sync.dma_start(out=hs_o.ap()[rows, :], in_=hs[:])
    nc.finalize()
    return nc


# --------------------------------------------------------------------------
# phase 2
# --------------------------------------------------------------------------

def _build_phase2(cfg, B, trivial_affine):
    H, HD, S, G, G2, NW = cfg.H, cfg.HD, cfg.S, cfg.G, cfg.G2, cfg.NW
    SW, WOFF = cfg.SW, cfg.WOFF
    assert B % G2 == 0 and B % G == 0
    GS = G * S
    nc = bacc.Bacc("TRN2", target_bir_lowering=False, debug=False,
                   num_devices=cfg.NC)
    tbl = nc.dram_tensor("tbl", [cfg.N_TBL, cfg.TROW], BF16,
                         kind="ExternalInput")
    sdst = [nc.dram_tensor("sdst_in", [cfg.NPC_PAD, 128], BF16,
                           kind="ExternalInput"),
            nc.dram_tensor("sdst_out", [cfg.NPC_PAD, 128], BF16,
                           kind="ExternalInput")]
    hself = nc.dram_tensor("hself", [cfg.NPC_PAD, cfg.OUT], F32,
                           kind="ExternalInput")
    midx = nc.dram_tensor("midx", [128, 2 * B * S * 8], I16,
                          kind="ExternalInput")
    sidx = nc.dram_tensor("sidx", [128, 2 * B * S * 8], I16,
                          kind="ExternalInput")
    slots = nc.dram_tensor("slots", [128, 2 * B * S], BF16,
                           kind="ExternalInput")
    hoidx = nc.dram_tensor("hoidx", [128, B * 8], I16, kind="ExternalInput")
    ooidx = nc.dram_tensor("ooidx", [128, B * 8], I16, kind="ExternalInput")
    gbb = nc.dram_tensor("gbb", [3, cfg.OUT], F32, kind="ExternalInput")
    out = nc.dram_tensor("out", [cfg.NPC_PAD, cfg.OUT], F32,
                         kind="ExternalOutput")

    # subtile -> (window, block-in-batch); first/last subtile per block
    sub_w = []
    sub_blk = []
    for s in range(GS):
        w = max(i for i in range(NW) if WOFF[i] * 128 <= s * 128)
        within = s - WOFF[w]
        sub_w.append(w)
        sub_blk.append(within // SW[w])
    first_s = [WOFF[0] + blk * SW[0] for blk in range(G)]
    last_s = [WOFF[NW - 1] + blk * SW[NW - 1] + SW[NW - 1] - 1
              for blk in range(G)]

    FW = H * (HD + 1)
    with TileContext(nc) as tc, ExitStack() as ctx:
        cpool = ctx.enter_context(tc.tile_pool(name="const", bufs=1))
        spool = ctx.enter_context(tc.tile_pool(name="stream", bufs=3))
        gpool = ctx.enter_context(tc.tile_pool(name="gather", bufs=2))
        wpool = ctx.enter_context(tc.tile_pool(name="work", bufs=2))
        fpool = ctx.enter_context(tc.tile_pool(name="fin", bufs=2))
        opool = ctx.enter_context(tc.tile_pool(name="ostage", bufs=2))
        ppool = ctx.enter_context(tc.tile_pool(name="psum", bufs=2,
                                               space="PSUM"))

        iota = cpool.tile([128, 128], BF16)
        nc.gpsimd.iota(iota[:], pattern=[[1, 128]], base=0,
                       channel_multiplier=0,
                       allow_small_or_imprecise_dtypes=True)
        eps_ln = cpool.tile([128, 1], F32)
        nc.gpsimd.memset(eps_ln[:], cfg.EPS_LN)
        ho_sb = cpool.tile([128, B * 8], I16)
        nc.sync.dma_start(out=ho_sb[:], in_=hoidx.ap())
        oo_sb = cpool.tile([128, B * 8], I16)
        nc.sync.dma_start(out=oo_sb[:], in_=ooidx.ap())
        if not trivial_affine:
            gam = cpool.tile([128, cfg.OUT], F32)
            nc.sync.dma_start(out=gam[:],
                              in_=gbb.ap()[0:1, :].to_broadcast(
                                  [128, cfg.OUT]))
            bet = cpool.tile([128, cfg.OUT], F32)
            nc.sync.dma_start(out=bet[:],
                              in_=gbb.ap()[1:2, :].to_broadcast(
                                  [128, cfg.OUT]))
            bia = cpool.tile([128, cfg.OUT], F32)
            nc.sync.dma_start(out=bia[:],
                              in_=gbb.ap()[2:3, :].to_broadcast(
                                  [128, cfg.OUT]))

        hself_g = None
        ostg = None
        n_batches = B // G
        for i in range(n_batches):
            b0 = i * G
            if b0 % G2 == 0:
                hself_g = opool.tile([128, G2, cfg.OUT], F32, tag="hgat")
                nc.gpsimd.dma_gather(hself_g[:], hself.ap(),
                                     ho_sb[:, b0 * 8:(b0 + G2) * 8],
                                     G2 * 128, G2 * 128, cfg.OUT,
                                     single_packet=(G2 * 128 <= 1024))
                ostg = opool.tile([128, G2, cfg.OUT], F32, tag="ostg")
            psd = [[None] * G, [None] * G]
            for d in range(2):
                base = (d * B * S + i * GS) * 8 * 16  # stream position
                mi = spool.tile([128, GS * 8], I16, tag="mi")
                nc.sync.dma_start(
                    out=mi[:], in_=midx.ap()[:, base // 16:
                                             base // 16 + GS * 8])
                si = spool.tile([128, GS * 8], I16, tag="si")
                nc.sync.dma_start(
                    out=si[:], in_=sidx.ap()[:, base // 16:
                                             base // 16 + GS * 8])
                sl = spool.tile([128, GS], BF16, tag="sl")
                nc.sync.dma_start(
                    out=sl[:], in_=slots.ap()[:, d * B * S + i * GS:
                                              d * B * S + (i + 1) * GS])

                gb = gpool.tile([128, GS, cfg.TROW], BF16, tag="gb")
                for w in range(NW):
                    nwi = G * SW[w] * 128
                    c0 = WOFF[w] * 128 // 16
                    nc.gpsimd.dma_gather(
                        gb[:, WOFF[w]:WOFF[w] + G * SW[w], :],
                        tbl.ap()[w * cfg.WROW:, :],
                        mi[:, c0:c0 + nwi // 16], nwi, nwi, cfg.TROW,
                        single_packet=(nwi <= 1024))
                sd = gpool.tile([128, GS, 128], BF16, tag="sd")
                nc.gpsimd.dma_gather(sd[:], sdst[d].ap(), si[:],
                                     GS * 128, GS * 128, 128,
                                     single_packet=(GS * 128 <= 1024))
                if _STAGE < 2:
                    continue

                e1 = wpool.tile([128, GS, H], BF16, tag="e1")
                nc.vector.tensor_tensor(out=e1[:],
                                        in0=gb[:, :, d * H:(d + 1) * H],
                                        in1=sd[:, :, 0:H], op=ALU.add)
                esc = wpool.tile([128, GS, H], BF16, tag="esc")
                nc.vector.tensor_scalar_mul(out=esc[:], in0=e1[:],
                                            scalar1=cfg.NEG)
                nc.vector.tensor_tensor(out=e1[:], in0=e1[:], in1=esc[:],
                                        op=ALU.max)
                alf = wpool.tile([128, GS, H], BF16, tag="alf")
                nc.scalar.activation(out=alf[:], in_=e1[:], func=AF.Exp)

                rhs = wpool.tile([128, GS, H, HD + 1], BF16, tag="rhs")
                gsl = gb[:, :, cfg.FCOL:cfg.FCOL + FW]
                gsl = gsl.rearrange("p s (h k) -> p s h k", h=H)
                ab = alf[:].unsqueeze(-1).to_broadcast([128, GS, H, HD + 1])
                nc.vector.tensor_tensor(out=rhs[:], in0=gsl, in1=ab,
                                        op=ALU.mult)
                M = wpool.tile([128, GS, 128], BF16, tag="M")
                ib = iota[:].unsqueeze(1).to_broadcast([128, GS, 128])
                sb = sl[:].unsqueeze(-1).to_broadcast([128, GS, 128])
                nc.vector.tensor_tensor(out=M[:], in0=ib, in1=sb,
                                        op=ALU.is_equal)
                if _STAGE < 3:
                    continue
                for blk in range(G):
                    ps_tile = ppool.tile([128, FW], F32, tag=f"ps{d}{blk}")
                    psd[d][blk] = ps_tile
                for s in range(GS):
                    blk = sub_blk[s]
                    nc.tensor.matmul(out=psd[d][blk][:], lhsT=M[:, s, :],
                                     rhs=rhs[:, s, :, :],
                                     start=(s == first_s[blk]),
                                     stop=(s == last_s[blk]))

            if _STAGE < 4:
                continue
            for blk in range(G):
                b = b0 + blk
                comb = fpool.tile([128, H, HD], F32, tag="comb")
                tdir = [None, None]
                for d in range(2):
                    ps = psd[d][blk]
                    psv = ps[:].rearrange("p (h k) -> p h k", h=H)
                    den = fpool.tile([128, H], F32, tag=f"den{d}")
                    nc.vector.tensor_scalar_add(out=den[:],
                                                in0=psv[:, :, HD:HD + 1],
                                                scalar1=cfg.EPS_SM)
                    rec = fpool.tile([128, H], F32, tag=f"rec{d}")
                    nc.vector.reciprocal(out=rec[:], in_=den[:])
                    td = fpool.tile([128, H, HD], F32, tag=f"td{d}")
                    rb = rec[:].unsqueeze(-1).to_broadcast([128, H, HD])
                    nc.vector.tensor_tensor(out=td[:], in0=psv[:, :, 0:HD],
                                            in1=rb, op=ALU.mult)
                    tdir[d] = td
                nc.vector.tensor_tensor(out=comb[:], in0=tdir[0][:],
                                        in1=tdir[1][:], op=ALU.add)
                combf = comb[:].rearrange("p h k -> p (h k)")
                nc.vector.tensor_tensor(out=combf, in0=combf,
                                        in1=hself_g[:, b % G2, :],
                                        op=ALU.add)
                if not trivial_affine:
                    nc.vector.tensor_tensor(out=combf, in0=combf,
                                            in1=bia[:], op=ALU.add)
                red = fpool.tile([128, 1], F32, tag="red")
                nc.vector.reduce_sum(out=red[:], in_=combf,
                                     axis=mybir.AxisListType.X)
                mean = fpool.tile([128, 1], F32, tag="mean")
                nc.vector.tensor_scalar_mul(out=mean[:], in0=red[:],
                                            scalar1=1.0 / cfg.OUT)
                xc = fpool.tile([128, cfg.OUT], F32, tag="xc")
                nc.vector.tensor_scalar_sub(out=xc[:], in0=combf,
                                            scalar1=mean[:])
                sq = fpool.tile([128, cfg.OUT], F32, tag="sq")
                nc.scalar.square(out=sq[:], in_=xc[:])
                v = fpool.tile([128, 1], F32, tag="v")
                nc.vector.reduce_sum(out=v[:], in_=sq[:],
                                     axis=mybir.AxisListType.X)
                lnv = fpool.tile([128, 1], F32, tag="lnv")
                nc.scalar.activation(out=lnv[:], in_=v[:], func=AF.Ln,
                                     scale=1.0 / cfg.OUT, bias=eps_ln[:])
                sv = fpool.tile([128, 1], F32, tag="sv")
                nc.scalar.activation(out=sv[:], in_=lnv[:], func=AF.Exp,
                                     scale=0.5)
                rstd = fpool.tile([128, 1], F32, tag="rstd")
                nc.vector.reciprocal(out=rstd[:], in_=sv[:])
                dst = ostg[:, b % G2, :]
                if trivial_affine:
                    nc.vector.tensor_scalar_mul(out=dst, in0=xc[:],
                                                scalar1=rstd[:])
                else:
                    nc.vector.tensor_scalar_mul(out=xc[:], in0=xc[:],
                                                scalar1=rstd[:])
                    nc.vector.tensor_tensor(out=xc[:], in0=xc[:],
                                            in1=gam[:], op=ALU.mult)
                    nc.vector.tensor_tensor(out=dst, in0=xc[:], in1=bet[:],
                                            op=ALU.add)
                if b % G2 == G2 - 1:
                    g0 = b - G2 + 1
                    nc.gpsimd.dma_scatter_add(
                        out.ap(), ostg[:],
                        oo_sb[:, g0 * 8:(g0 + G2) * 8],
                        G2 * 128, G2 * 128, cfg.OUT,
                        single_packet=(G2 * 128 <= 1024))
    nc.finalize()
    return nc


# --------------------------------------------------------------------------
# driver
# --------------------------------------------------------------------------

def _get_phase1(cfg):
    key = ("p1", cfg.N, cfg.E, cfg.NC, cfg.S)
    if key not in _PROG_CACHE:
        _PROG_CACHE[key] = _build_phase1(cfg)
    return _PROG_CACHE[key]


def _get_phase2(cfg, B, trivial):
    key = ("p2", cfg.N, cfg.E, cfg.NC, cfg.S, B, trivial)
    if key not in _PROG_CACHE:
        _PROG_CACHE[key] = _build_phase2(cfg, B, trivial)
    return _PROG_CACHE[key]


def run(cfg, h, edge_index, W, W_self, a_in, a_out, bias, ln_gamma, ln_beta,
        timing=None):
    h = np.asarray(h, np.float32)
    edge_index = np.asarray(edge_index)
    W = np.asarray(W, np.float32)
    W_self = np.asarray(W_self, np.float32)
    a_in = np.asarray(a_in, np.float32)
    a_out = np.asarray(a_out, np.float32)
    bias = np.asarray(bias, np.float32)
    ln_gamma = np.asarray(ln_gamma, np.float32)
    ln_beta = np.asarray(ln_beta, np.float32)

    wa = _build_wa(cfg, W, W_self, a_in, a_out)
    core_ids = list(range(cfg.NC))

    nc1 = _get_phase1(cfg)
    in1 = []
    for c in range(cfg.NC):
        shard = np.zeros((cfg.NPC_PAD, cfg.IN), np.float32)
        shard[:cfg.NPC] = h[c * cfg.NPC:(c + 1) * cfg.NPC]
        in1.append({"h": shard, "wa": wa})
    r1 = run_bass_kernel_spmd(nc1, in1, core_ids)
    res1 = r1.results

    table = np.concatenate([res1[c]["tbl"] for c in range(cfg.NC)], axis=0)

    B, streams = _preprocess(cfg, edge_index)

    trivial = (not bias.any()) and (ln_gamma == 1.0).all() \
        and (not ln_beta.any())
    gbb = np.stack([ln_gamma, ln_beta, bias]).astype(np.float32)

    nc2 = _get_phase2(cfg, B, trivial)
    in2 = []
    for c in range(cfg.NC):
        s = streams[c]
        in2.append({
            "tbl": table,
            "sdst_in": res1[c]["sdst_in"],
            "sdst_out": res1[c]["sdst_out"],
            "hself": res1[c]["hself"],
            "midx": s["midx"], "sidx": s["sidx"], "slots": s["slots"],
            "hoidx": s["hoidx"], "ooidx": s["ooidx"],
            "gbb": gbb,
        })
    r2 = run_bass_kernel_spmd(nc2, in2, core_ids)
    res2 = r2.results

    out = np.concatenate(
        [res2[c]["out"][:cfg.NPC] for c in range(cfg.NC)], axis=0)
    if timing is not None:
        timing["exec_ns"] = [r1.exec_time_ns, r2.exec_time_ns]
    return out.astype(np.float32)


def kernel(h, edge_index, W, W_self, a_in, a_out, bias, ln_gamma, ln_beta):
    return run(DEFAULT_CFG, h, edge_index, W, W_self, a_in, a_out, bias,
               ln_gamma, ln_beta)
